# revision 5
# baseline (speedup 1.0000x reference)
"""Trainium2 Bass kernel for the Correlation module.

reference:
    affinities = einsum('lnd,ond->lon', x, upfold) / sqrt(d)   # [L,O,N]
    features   = einsum('lon,ond->lnd', sigmoid(affinities)-0.5, upfold)

Math used here: sigmoid(a)-0.5 = 0.5*tanh(a/2), so with s = 1/sqrt(64):
    W^T = tanh(A^T / 16)            (A = x @ upfold^T per n)
    F   = W @ (0.5*upfold)          (0.5 folded into the mm2 stationary)

The matmul datapath runs in bf16: f32r matmuls execute in fp32-HIGH mode
on TRN2 (~3x slower per row), while bf16 streams at 1 row/cycle. GpSimd
casts the raw f32 input tiles to bf16 so the PE transposes run at bf16
rate too; rel-err budget (2e-2) dwarfs bf16 rounding.

Sharding: data-parallel over N across 8 cores (8 n per core), processed
as 4 pairs of n so that PE row tiling packs two K=64 matmuls (mm1) into
the full 128-row array. mm2 runs per-n with M=64 into separate PSUM
accumulators via zero-padded stationaries.

Self-contained: hardcodes shapes; no reads of /root/problem/*.
"""

import numpy as np

L, N, D, O = 1024, 64, 64, 1024
NCORES = 8
NLOC = N // NCORES   # 8 n per core
NPAIRS = NLOC // 2   # 4 pairs

_CACHE = {}


def _build_program():
    import concourse.mybir as mybir
    import concourse.tile as tile
    from concourse import bacc
    from concourse.masks import make_identity

    f32 = mybir.dt.float32
    f32r = mybir.dt.float32r
    bf16 = mybir.dt.bfloat16
    fp8 = mybir.dt.float8e4
    DR = mybir.MatmulPerfMode.DoubleRow
    TANH = mybir.ActivationFunctionType.Tanh

    nc = bacc.Bacc(
        "TRN2", target_bir_lowering=False, debug=False, num_devices=NCORES
    )
    x_ap = nc.dram_tensor("x", [L, NLOC, D], f32r, kind="ExternalInput").ap()
    u_ap = nc.dram_tensor("upfold", [O, NLOC, D], f32r, kind="ExternalInput").ap()
    o_ap = nc.dram_tensor("out", [L, NLOC, D], f32, kind="ExternalOutput").ap()

    with tile.TileContext(nc) as tc:
        with (
            tc.tile_pool(name="const", bufs=1) as constp,
            tc.tile_pool(name="io", bufs=2) as iop,
            tc.tile_pool(name="bfp", bufs=2) as bfp,
            tc.tile_pool(name="tsp", bufs=2) as tsp,
            tc.tile_pool(name="wt", bufs=3) as wtp,
            tc.tile_pool(name="fsb", bufs=2) as fsbp,
            tc.tile_pool(name="ost", bufs=2) as ostp,
            tc.tile_pool(name="uzp", bufs=2) as uzp,
            tc.tile_pool(name="atps", bufs=2, space="PSUM") as atps,
            tc.tile_pool(name="ftps", bufs=1, space="PSUM") as ftps,
            tc.tile_pool(name="trps", bufs=2, space="PSUM") as trps,
        ):
            ident_f = constp.tile([128, 128], f32)
            make_identity(nc, ident_f[:])
            ident = constp.tile([128, 128], bf16)
            nc.vector.tensor_copy(ident[:], ident_f[:])

            loaded = {}
            staged = {}

            def load_pair(p):
                """Half-granular DMAs so casts/transposes can start earlier."""
                n0 = 2 * p
                halves = []
                for src_ap, tagb in ((x_ap, "xp"), (u_ap, "up")):
                    full = src_ap[:, n0 : n0 + 2, :].rearrange(
                        "(lc q) n d -> q lc (n d)", q=128
                    )
                    ha = iop.tile([128, 4, 128], f32r, tag=tagb + "a")
                    nc.sync.dma_start(ha[:], full[:, 0:4, :])
                    hb = iop.tile([128, 4, 128], f32r, tag=tagb + "b")
                    nc.sync.dma_start(hb[:], full[:, 4:8, :])
                    halves.extend([ha, hb])
                staged[p] = halves

            deferred = {}

            def prep_pair(p, defer_tail=False):
                """Cast to bf16, then build XT/UT ([d-pair, l/o]) and the
                0.5-scaled zero-padded mm2 stationaries."""
                xpa, xpb, upa, upb = staged.pop(p)
                # GpSimd casts f32 -> bf16 (engine otherwise idle)
                casts = []
                for src, tag in ((xpa, "xba"), (xpb, "xbb"), (upa, "uba"), (upb, "ubb")):
                    b = bfp.tile([128, 4, 128], bf16, tag=tag)
                    nc.gpsimd.tensor_copy(b[:], src[:])
                    casts.append(b)
                xba, xbb, uba, ubb = casts

                XT = tsp.tile([128, 1024], bf16, tag="XT")
                UT = tsp.tile([128, 1024], bf16, tag="UT")

                def tr_group(dst, half, g):
                    tp = trps.tile([128, 512], bf16, tag="tp")
                    for j in range(4):
                        nc.tensor.transpose(
                            tp[:, 128 * j : 128 * (j + 1)], half[:, j, :], ident[:]
                        )
                    nc.vector.tensor_copy(dst[:, 512 * g : 512 * (g + 1)], tp[:])

                tr_group(XT, xba, 0)
                tr_group(UT, uba, 0)
                tr_group(XT, xbb, 1)
                if defer_tail:
                    # UT[:, 512:] is first needed at oc=4 -- keep the slow
                    # last chain off the PE FIFO ahead of the first matmuls
                    deferred[p] = lambda: tr_group(UT, ubb, 1)
                else:
                    tr_group(UT, ubb, 1)
                # uza = [0.5*U_n1 | 0], uzb = [0 | 0.5*U_n2] per o-chunk
                # (fp8e4: mm2 runs in DoubleRow mode, 2 o-chunks per pass)
                uza = uzp.tile([128, 8, 128], fp8, tag="uza")
                uzb = uzp.tile([128, 8, 128], fp8, tag="uzb")
                for g, uh in enumerate((uba, ubb)):
                    s = slice(4 * g, 4 * g + 4)
                    nc.vector.tensor_scalar_mul(uza[:, s, 0:64], uh[:, :, 0:64], 0.5)
                    nc.vector.tensor_scalar_mul(uza[:, s, 64:128], uh[:, :, 64:128], 0.0)
                    nc.vector.tensor_scalar_mul(uzb[:, s, 64:128], uh[:, :, 64:128], 0.5)
                    nc.vector.tensor_scalar_mul(uzb[:, s, 0:64], uh[:, :, 0:64], 0.0)
                loaded[p] = (XT, UT, uza, uzb)

            def emit_out(p, fsb):
                # F-transposes (bf16) + f32 store, pipelined by halves
                n0 = 2 * p
                dst = o_ap[:, n0 : n0 + 2, :].rearrange(
                    "(lc q) n d -> q lc (n d)", q=128
                )
                for g in range(2):
                    tp = trps.tile([128, 512], bf16, tag="tp")
                    for j in range(4):
                        lc = 4 * g + j
                        nc.tensor.transpose(
                            tp[:, 128 * j : 128 * (j + 1)],
                            fsb[:, 128 * lc : 128 * (lc + 1)],
                            ident[:],
                        )
                    ost = ostp.tile([128, 512], f32, tag="ost")
                    nc.vector.tensor_copy(ost[:], tp[:])
                    nc.sync.dma_start(
                        dst[:, 4 * g : 4 * g + 4, :],
                        ost[:].rearrange("q (lc nd) -> q lc nd", nd=128),
                    )

            def oc_loop(p, carry):
                XT, UT, uza, uzb = loaded.pop(p)
                ft = ftps.tile([128, 1024], f32, tag="ft")

                def mm1_half(oc, ni, at):
                    rows = slice(64 * ni, 64 * (ni + 1))
                    for lh in range(2):
                        nc.tensor.matmul(
                            at[:, 512 * lh : 512 * (lh + 1)],
                            UT[rows, 128 * oc : 128 * (oc + 1)],
                            XT[rows, 512 * lh : 512 * (lh + 1)],
                            start=True,
                            stop=True,
                            tile_position=(64 * ni, 0),
                        )

                def mm2_dr(ocp, ni, w):
                    # DoubleRow: contracts 2 o-chunks (256 rows) per pass
                    uzt = uza if ni == 0 else uzb
                    for lh in range(2):
                        nc.tensor.matmul(
                            ft[:, 512 * lh : 512 * (lh + 1)],
                            uzt[:, 2 * ocp : 2 * ocp + 2, :],
                            w[:, :, 512 * lh : 512 * (lh + 1)],
                            start=(ocp == 0 and ni == 0),
                            stop=(ocp == 3 and ni == 1),
                            perf_mode=DR,
                        )

                prev = None  # (ocp, wA, wB) awaiting mm2
                wA = wB = None
                tail = deferred.pop(p, None)
                pending = None
                for oc in range(8):
                    if oc == 1 and tail is not None:
                        tail()
                    if oc == 1 and carry is not None:
                        pending = carry["fsb"]()
                    if oc == 3 and pending is not None:
                        emit_out(*pending)
                    if oc == 4 and p + 1 < NPAIRS:
                        load_pair(p + 1)
                    if oc == 5 and p + 1 < NPAIRS:
                        prep_pair(p + 1)
                    at0 = atps.tile([128, 1024], f32, tag="at")
                    at1 = atps.tile([128, 1024], f32, tag="at")
                    # PE stream: mm1(at0) | mm2(prev) | mm1(at1) | mm2(prev)
                    mm1_half(oc, 0, at0)
                    if oc % 2 == 0:
                        if prev is not None:
                            mm2_dr(prev[0], 0, prev[1])
                        elif oc == 0 and carry is not None:
                            carry["mm2a"]()
                    mm1_half(oc, 1, at1)
                    if oc % 2 == 0:
                        if prev is not None:
                            mm2_dr(prev[0], 1, prev[2])
                        elif oc == 0 and carry is not None:
                            carry["mm2b"]()
                    if oc % 2 == 0:
                        wA = wtp.tile([128, 2, 1024], fp8, tag="w0")
                        wB = wtp.tile([128, 2, 1024], fp8, tag="w1")
                    k = oc % 2
                    nc.scalar.activation(wA[:, k, :], at0[:], TANH, scale=1.0 / 16.0)
                    nc.scalar.activation(wB[:, k, :], at1[:], TANH, scale=1.0 / 16.0)
                    if oc % 2 == 1:
                        prev = (oc // 2, wA, wB)

                def make_fsb():
                    fsb = fsbp.tile([128, 1024], bf16, name="fsb")
                    nc.vector.tensor_copy(fsb[:], ft[:])
                    return (p, fsb)

                return {
                    "mm2a": lambda: mm2_dr(3, 0, prev[1]),
                    "mm2b": lambda: mm2_dr(3, 1, prev[2]),
                    "fsb": make_fsb,
                }

            load_pair(0)
            prep_pair(0)
            carry = None
            for p in range(NPAIRS):
                carry = oc_loop(p, carry)
            carry["mm2a"]()
            carry["mm2b"]()
            emit_out(*carry["fsb"]())

    nc.compile()
    return nc


def _get_program():
    if "nc" not in _CACHE:
        _CACHE["nc"] = _build_program()
    return _CACHE["nc"]


def _make_in_maps(x, upfold):
    x = np.asarray(x, dtype=np.float32)
    upfold = np.asarray(upfold, dtype=np.float32)
    in_maps = []
    for c in range(NCORES):
        s = slice(NLOC * c, NLOC * (c + 1))
        in_maps.append(
            {
                "x": np.ascontiguousarray(x[:, s, :]),
                "upfold": np.ascontiguousarray(upfold[:, s, :]),
            }
        )
    return in_maps


def run_sharded(x, upfold, trace=False, **kwargs):
    """Run on all 8 cores; returns (full_output, BassKernelResults)."""
    from concourse.bass_utils import run_bass_kernel_spmd

    nc = _get_program()
    res = run_bass_kernel_spmd(
        nc, _make_in_maps(x, upfold), core_ids=list(range(NCORES)),
        trace=trace, **kwargs
    )
    out = np.concatenate([res.results[c]["out"] for c in range(NCORES)], axis=1)
    return out, res


def kernel(x, upfold):
    out, _ = run_sharded(x, upfold)
    return out
